# revision 34
# baseline (speedup 1.0000x reference)
"""Single-head attention (B=8, S=2048, H=768, D=64) on 8 TRN2 NeuronCores.

Data-parallel over batch: core b computes batch element b end to end; no
collectives. Host pre-transposes Q/K/V to [H, S] bf16 so every matmul
contraction lands on the partition axis.

Per-core dataflow (matmuls bf16 x bf16 -> f32 PSUM):
  warmup MMs on the identity flip the PE HAM clock-gate to 2.4 GHz while
    the first input quarters stream in.
  [qT; kT][128, 512] = [Wq|Wk]^T @ [queryT, keyT]  per 512-col quarter via
    col-packed concurrent matmul pairs; bias, then PE identity-matmuls
    duplicate qT/kT into both partition halves (qqT/kkT) so scores can
    alternate PE row groups (hides LDWEIGHTS).
  S^T tile [sk=128, sq=1024] per (t, half) in PSUM; P^T = exp(S^T/8 + mask)
    straight to bf16 SBUF (ACT engine saturated ~35us = the floor).
  O^T accumulation: per (t, half) a col-packed concurrent pair
    (M=64 each, cols 0-63 / 64-127) multiplies V^T_t against the two
    512-col chunks of P^T; denominators via 4x col-packed ones[128,32]
    matmuls accumulated over t.
  Unnormalized O^T chunks + denominator rows DMA out as [128, 1536] f32;
  host does the divide + transpose (cheap numpy).
"""

import os
from contextlib import ExitStack

import numpy as np
import ml_dtypes

import concourse.bass as bass
import concourse.mybir as mybir
import concourse.tile as tile
from concourse import bacc
from concourse.bass_utils import run_bass_kernel_spmd

S, H, D = 2048, 768, 64
P = 128
NT = S // P      # 16 sk tiles
HT = H // P      # 6 h tiles
CH = 512         # sq chunk (quarter) = matmul free dim = PSUM bank
NCH = S // CH    # 4
BF = mybir.dt.bfloat16
F32 = mybir.dt.float32
AF = mybir.ActivationFunctionType

LAST_RESULT = None  # BassKernelResults of the most recent run (for test.py)


def _build(debug=False):
    nc = bacc.Bacc()
    qT_d = nc.declare_dram_parameter("qT", [H, S], BF, isOutput=False)
    kT_d = nc.declare_dram_parameter("kT", [H, S], BF, isOutput=False)
    vT_d = nc.declare_dram_parameter("vT", [H, S], BF, isOutput=False)
    # {ident | wqk | wvv} host-prepacked into one [p, n] tensor so a single
    # contiguous DMA delivers them (each dma_start costs ~680ns of serial
    # descriptor-gen on its queue's sequencer)
    wid_d = nc.declare_dram_parameter("wid", [P, P + 2 * HT * P], BF, isOutput=False)
    # {bqk | bvv | mb} packed the same way
    cst_d = nc.declare_dram_parameter("cst", [P, 4 + NT], F32, isOutput=False)
    o_d = nc.declare_dram_parameter("o", [P, 3 * CH], F32, isOutput=True)

    with ExitStack() as ctx:
        tc = ctx.enter_context(tile.TileContext(nc))
        consts = ctx.enter_context(tc.tile_pool(name="consts", bufs=1))
        stqk = ctx.enter_context(tc.tile_pool(name="stqk", bufs=6))
        stv = ctx.enter_context(tc.tile_pool(name="stv", bufs=4))
        persist = ctx.enter_context(tc.tile_pool(name="persist", bufs=1))
        ppool = ctx.enter_context(tc.tile_pool(name="ppool", bufs=2 * NT))
        psc = ctx.enter_context(tc.tile_pool(name="psc", bufs=1, space="PSUM"))
        psw = ctx.enter_context(tc.tile_pool(name="psw", bufs=1, space="PSUM"))
        psav = ctx.enter_context(tc.tile_pool(name="psav", bufs=1, space="PSUM"))

        # ---- constants: one DMA each ----
        wid_sb = consts.tile([P, P + 2 * HT * P], BF, tag="wid")
        nc.scalar.dma_start(out=wid_sb, in_=wid_d[:, :])
        ident_bf = wid_sb[:, 0:P]

        def w_half(kind, h, lo, hi):  # packed weight slice [128, hi-lo]
            base = P + kind * HT * P + h * P
            return wid_sb[:, base + lo : base + hi]

        cst_sb = consts.tile([P, 4 + NT], F32, tag="cst")
        nc.scalar.dma_start(out=cst_sb, in_=cst_d[:, :])
        bqk_sb = cst_sb[:, 0:1]
        bvv_sb = cst_sb[:, 1:2]
        bkk_sb = cst_sb[:, 2:3]
        bqq_sb = cst_sb[:, 3:4]
        mb_sb = cst_sb[:, 4 : 4 + NT]
        ones32 = consts.tile([P, 32], BF, tag="ones32")
        nc.vector.memset(ones32, 1.0)

        # ---- input staging: whole halves as single rearranged DMAs
        # (768 descriptors x 2KB lines each, streaming while generating).
        # Queue plan keeps every transfer off the ACT queue except q-half0
        # (whose trigger completes before the first exp could run anyway):
        #   sync:   wid, k-h0, k-h1, q-h1, v[h0-2]-h0, v[h0-2]-h1, outputs
        #   scalar: q-h0, then the exp stream
        #   gpsimd: cst, v[h3-5]-h0, v[h3-5]-h1
        st_q, st_k, st_v = {}, {}, {}
        HS = S // 2
        kst, qst, vst_a, vst_b = {}, {}, {}, {}
        # The ACT (scalar) queue starts issuing at ~2.7us while the SP
        # queue pays ~7us of framework init. So: the critical k0/q0 ride
        # the scalar queue (split in h-thirds so projection starts on
        # partially-arrived data), everything else on the SP queue.
        crit = {}
        for nm, src in (("k0", kT_d), ("q0", qT_d)):
            for part in range(2):
                t3 = stqk.tile([P, 3, HS], BF, tag="stqk", name=f"{nm}_{part}")
                crit[nm, part] = t3
        for nm, src in (("k0", kT_d), ("q0", qT_d)):
            for part in range(2):
                nc.sync.dma_start(
                    out=crit[nm, part],
                    in_=src[3 * part * P : 3 * (part + 1) * P, 0:HS].rearrange(
                        "(t p) n -> p t n", p=P
                    ),
                )
        kst[1] = stqk.tile([P, HT, HS], BF, tag="stqk", name="st_k1")
        nc.sync.dma_start(
            out=kst[1], in_=kT_d[:, HS : 2 * HS].rearrange("(t p) n -> p t n", p=P)
        )
        qst[1] = stqk.tile([P, HT, HS], BF, tag="stqk", name="st_q1")
        nc.sync.dma_start(
            out=qst[1], in_=qT_d[:, HS : 2 * HS].rearrange("(t p) n -> p t n", p=P)
        )
        for hf in range(2):
            vst_a[hf] = stv.tile([P, 3, HS], BF, tag="stv", name=f"st_va{hf}")
            vst_b[hf] = stv.tile([P, 3, HS], BF, tag="stv", name=f"st_vb{hf}")
        for hf in range(2):
            nc.sync.dma_start(
                out=vst_a[hf],
                in_=vT_d[0 : 3 * P, hf * HS : (hf + 1) * HS].rearrange(
                    "(t p) n -> p t n", p=P
                ),
            )
            nc.sync.dma_start(
                out=vst_b[hf],
                in_=vT_d[3 * P : 6 * P, hf * HS : (hf + 1) * HS].rearrange(
                    "(t p) n -> p t n", p=P
                ),
            )
        for c in range(NCH):
            for h in range(HT):
                if c < 2:
                    st_k[h, c] = crit["k0", h // 3][
                        :, h % 3, c * CH : (c + 1) * CH
                    ]
                    st_q[h, c] = crit["q0", h // 3][
                        :, h % 3, c * CH : (c + 1) * CH
                    ]
                else:
                    st_k[h, c] = kst[1][:, h, (c % 2) * CH : (c % 2 + 1) * CH]
                    st_q[h, c] = qst[1][:, h, (c % 2) * CH : (c % 2 + 1) * CH]
        for half in range(2):
            for h in range(HT):
                grp = vst_a if h < 3 else vst_b
                for i in range(2):
                    st_v[h, half, i] = grp[half][
                        :, h % 3, i * CH : (i + 1) * CH
                    ]

        # ---- persistent SBUF tensors ----
        qqT_sb = persist.tile([P, S], BF, tag="qqT")  # qT in both halves
        kkT_sb = persist.tile([P, S], BF, tag="kkT")  # kT in both halves
        vT2_sb = persist.tile([P, S // 2], BF, tag="vT2")  # vT chunk pairs
        vE_sb = persist.tile([P, NT * D], BF, tag="vE")  # V tiles [sk, d]
        osb = persist.tile([P, 3 * CH], F32, tag="osb")  # output staging

        # ---- PE warmup: back-to-back dummy matmuls flip the HAM clock
        # gate to 2.4 GHz during the DMA lead-in (~3.4us of activity) ----
        warm = psw.tile([P, CH], F32, tag="aux", name="warm")
        for i in range(30):
            nc.tensor.matmul(
                warm[:, :P],
                lhsT=ident_bf,
                rhs=ident_bf,
                start=True,
                stop=True,
                skip_group_check=True,
            )

        # ---- helper blocks ----
        def qk_proj_chunk(c):
            """col-packed concurrent q/k projection for sq quarter c, then
            PE identity-matmul partition duplication for that quarter."""
            pp = psav.tile(
                [P, CH], F32, tag="av01" if c < 2 else "av23",
                name=f"pp{c}",
            )
            for h in range(HT):
                nc.tensor.matmul(
                    pp[:D, :],
                    lhsT=w_half(0, h, 0, D),
                    rhs=st_q[h, c],
                    start=(h == 0),
                    stop=(h == HT - 1),
                    tile_position=(0, 0),
                    skip_group_check=True,
                )
                nc.tensor.matmul(
                    pp[D:, :],
                    lhsT=w_half(0, h, D, P),
                    rhs=st_k[h, c],
                    start=(h == 0),
                    stop=(h == HT - 1),
                    tile_position=(0, D),
                    skip_group_check=True,
                )
            ch = slice(c * CH, (c + 1) * CH)
            nc.vector.tensor_scalar_add(
                out=qqT_sb[:D, ch], in0=pp[:D, :], scalar1=bqk_sb[:D, :]
            )
            nc.vector.tensor_scalar_add(
                out=kkT_sb[D:, ch], in0=pp[D:, :], scalar1=bqk_sb[D:, :]
            )
            pd = psw.tile([P, CH], F32, tag="aux" if c % 2 else "den",
                          name=f"pd{c}")
            nc.tensor.matmul(
                pd[D:, :],
                lhsT=ident_bf[:D, :D],
                rhs=qqT_sb[:D, ch],
                start=True,
                stop=True,
                tile_position=(0, D),
                skip_group_check=True,
            )
            nc.tensor.matmul(
                pd[:D, :],
                lhsT=ident_bf[D:, D:],
                rhs=kkT_sb[D:, ch],
                start=True,
                stop=True,
                tile_position=(D, 0),
                skip_group_check=True,
            )
            nc.vector.tensor_copy(out=qqT_sb[D:, ch], in_=pd[D:, :])
            nc.vector.tensor_copy(out=kkT_sb[:D, ch], in_=pd[:D, :])

        def kq_proj_23(kind):
            """chunks 2,3 projection for one input only: kind 0 = k
            (needs just k-half1), kind 1 = q. Two col-packed concurrent
            pairs produce the native and duplicated partition halves
            directly."""
            dst = kkT_sb if kind == 0 else qqT_sb
            bias = bkk_sb if kind == 0 else bqq_sb
            wlo = D if kind == 0 else 0
            stx = st_k if kind == 0 else st_q
            for pidx in range(2):
                cA = 2 + pidx
                cB = 2 + (1 - pidx)
                pp = psav.tile([P, CH], F32, tag="av23",
                               name=f"pp23_{kind}_{pidx}")
                for h in range(HT):
                    nc.tensor.matmul(
                        pp[:D, :],
                        lhsT=w_half(0, h, wlo, wlo + D),
                        rhs=stx[h, cA],
                        start=(h == 0),
                        stop=(h == HT - 1),
                        tile_position=(0, 0),
                        skip_group_check=True,
                    )
                    nc.tensor.matmul(
                        pp[D:, :],
                        lhsT=w_half(0, h, wlo, wlo + D),
                        rhs=stx[h, cB],
                        start=(h == 0),
                        stop=(h == HT - 1),
                        tile_position=(0, D),
                        skip_group_check=True,
                    )
                nc.vector.tensor_scalar_add(
                    out=dst[:D, cA * CH : (cA + 1) * CH],
                    in0=pp[:D, :],
                    scalar1=bias[:D, :],
                )
                nc.vector.tensor_scalar_add(
                    out=dst[D:, cB * CH : (cB + 1) * CH],
                    in0=pp[D:, :],
                    scalar1=bias[D:, :],
                )

        pth = {}

        def scores_exp(t, half):
            """scores for sk-tile t over sq half (row group alternates with
            t to hide LDWEIGHTS), exp straight into a bf16 tile."""
            lo, hi = (0, D) if t % 2 == 0 else (D, P)
            ps = psc.tile([P, 2 * CH], F32, tag=f"sc{t % 2}",
                          name=f"ps{t}_{half}")
            for sub in range(2):
                c = 2 * half + sub
                nc.tensor.matmul(
                    ps[:, sub * CH : (sub + 1) * CH],
                    lhsT=kkT_sb[lo:hi, t * P : (t + 1) * P],
                    rhs=qqT_sb[lo:hi, c * CH : (c + 1) * CH],
                    start=True,
                    stop=True,
                    tile_position=(lo, 0),
                    skip_group_check=True,
                )
            pt = ppool.tile([P, 2 * CH], BF, tag="pT", name=f"pt{t}_{half}")
            nc.scalar.activation(
                out=pt,
                in_=ps,
                func=AF.Exp,
                bias=mb_sb[:, t : t + 1],
                scale=0.125,
            )
            pth[t, half] = pt

        pav = {}

        def vE_slice(t):
            """vE block layout: transpose j holds tiles (k, k+4) side by
            side; see v_proj."""
            b = (t // 8) * 4 + (t % 4)
            half = (t % 8) // 4
            return vE_sb[:, b * P + half * D : b * P + half * D + D]

        def av(t, half):
            """col-packed concurrent O^T accumulation pair for sk-tile t:
            chunk 2*half -> partitions 0:64, chunk 2*half+1 -> 64:128."""
            key = f"av{2 * half}{2 * half + 1}"
            if half not in pav:
                pav[half] = psav.tile([P, CH], F32, tag=key, name=key)
            vt = vE_slice(t)
            nc.tensor.matmul(
                pav[half][:D, :],
                lhsT=vt,
                rhs=pth[t, half][:, :CH],
                start=(t == 0),
                stop=(t == NT - 1),
                tile_position=(0, 0),
                skip_group_check=True,
            )
            nc.tensor.matmul(
                pav[half][D:, :],
                lhsT=vt,
                rhs=pth[t, half][:, CH:],
                start=(t == 0),
                stop=(t == NT - 1),
                tile_position=(0, D),
                skip_group_check=True,
            )

        pden = [None]

        def den4(t):
            """4x col-packed concurrent denominator matmuls: chunk ci's
            softmax denominator accumulates in partitions 32ci:32ci+32."""
            if pden[0] is None:
                pden[0] = psw.tile([P, CH], F32, tag="den", name="pden")
            for ci in range(NCH):
                nc.tensor.matmul(
                    pden[0][32 * ci : 32 * (ci + 1), :],
                    lhsT=ones32[:, :],
                    rhs=pth[t, ci // 2][:, (ci % 2) * CH : (ci % 2 + 1) * CH],
                    start=(t == 0),
                    stop=(t == NT - 1),
                    tile_position=(0, 32 * ci),
                    skip_group_check=True,
                )

        def v_proj(u):
            """v projection chunk-pair u (chunks 2u -> rows 0:64,
            2u+1 -> rows 64:128), then PE-transposes [128,128] blocks of
            vT2 into vE (each block = vE tiles k and k+4 side by side)."""
            pv = psw.tile([P, CH], F32, tag="den" if u == 0 else "aux",
                          name=f"pv{u}")
            for h in range(HT):
                nc.tensor.matmul(
                    pv[:D, :],
                    lhsT=w_half(1, h, 0, D),
                    rhs=st_v[h, u, 0],
                    start=(h == 0),
                    stop=(h == HT - 1),
                    tile_position=(0, 0),
                    skip_group_check=True,
                )
                nc.tensor.matmul(
                    pv[D:, :],
                    lhsT=w_half(1, h, D, P),
                    rhs=st_v[h, u, 1],
                    start=(h == 0),
                    stop=(h == HT - 1),
                    tile_position=(0, D),
                    skip_group_check=True,
                )
            nc.vector.tensor_scalar_add(
                out=vT2_sb[:, u * CH : (u + 1) * CH], in0=pv, scalar1=bvv_sb
            )
            for j in range(4):
                pt = psw.tile([P, P], BF, tag="aux", name=f"ptv{u}_{j}")
                nc.tensor.transpose(
                    pt,
                    in_=vT2_sb[:, u * CH + j * P : u * CH + (j + 1) * P],
                    identity=ident_bf,
                )
                b = 4 * u + j
                nc.vector.tensor_copy(
                    out=vE_sb[:, b * P : (b + 1) * P], in_=pt
                )

        # ---- schedule (program order == Tile priority) ----
        # phase A: project q/k quarters, stream scores+exp for sq half 0,
        # slot v-projection + first-half AV under the exp umbrella.
        # phase A: the entire first-half score/exp stream runs with NO
        # v- or half1-dependent instruction ahead of it in any engine
        # queue (PE FIFO head-of-line blocking otherwise stalls the ACT
        # chain on DMA arrivals).
        qk_proj_chunk(0)
        qk_proj_chunk(1)
        for t in range(8):
            scores_exp(t, 0)
        kq_proj_23(0)
        for t in range(8, NT):
            scores_exp(t, 0)
        kq_proj_23(1)
        v_proj(0)
        for t in range(8):
            av(t, 0)

        # phase B: sq half 1 + denominators in coarse PE bursts; the
        # v-half1-dependent work slots in after the first two pairs.
        def pair_b(tp):
            scores_exp(tp, 1)
            scores_exp(tp + 1, 1)
            av(tp, 1)
            den4(tp)
            av(tp + 1, 1)
            den4(tp + 1)

        pair_b(0)
        pair_b(2)
        v_proj(1)
        for t in range(8, NT):
            av(t, 0)
        for tp in range(4, NT, 2):
            pair_b(tp)

        # ---- epilogue: stage unnormalized O^T + denominators, DMA out;
        # the host divides and transposes ----
        nc.vector.tensor_copy(out=osb[:, 0:CH], in_=pav[0])
        nc.sync.dma_start(out=o_d[:, 0:CH], in_=osb[:, 0:CH])
        nc.vector.tensor_copy(out=osb[:, CH : 2 * CH], in_=pav[1])
        nc.vector.tensor_copy(out=osb[:, 2 * CH : 3 * CH], in_=pden[0])
        nc.sync.dma_start(out=o_d[:, CH : 3 * CH], in_=osb[:, CH : 3 * CH])

    return nc


_NC = None


def kernel(query, key, value, mask, Wq, bq, Wk, bk, Wv, bv):
    global _NC, LAST_RESULT
    bf16 = ml_dtypes.bfloat16
    B = query.shape[0]
    assert B == 8

    if _NC is None:
        _NC = _build()
        _NC.finalize()  # run bacc passes (wait splitting, reg alloc, ACT tables)

    def prepack(w):  # [768, 128] -> [p, t, n] layout [128, 768]
        return np.ascontiguousarray(
            w.reshape(HT, P, P).transpose(1, 0, 2).reshape(P, HT * P).astype(bf16)
        )

    wid = np.ascontiguousarray(
        np.concatenate(
            [
                np.eye(P, dtype=bf16),
                prepack(np.concatenate([np.asarray(Wq), np.asarray(Wk)], axis=1)),
                prepack(np.concatenate([np.asarray(Wv), np.asarray(Wv)], axis=1)),
            ],
            axis=1,
        )
    )
    bqk = np.concatenate([np.asarray(bq), np.asarray(bk)]).astype(np.float32)
    bvv = np.concatenate([np.asarray(bv), np.asarray(bv)]).astype(np.float32)

    in_maps = []
    for b in range(B):
        mb = ((np.asarray(mask[b], np.float32) - 1.0) * 1e9).reshape(NT, P).T
        bkk = np.concatenate([np.asarray(bk), np.asarray(bk)]).astype(np.float32)
        bqq = np.concatenate([np.asarray(bq), np.asarray(bq)]).astype(np.float32)
        cst = np.ascontiguousarray(
            np.concatenate(
                [bqk[:, None], bvv[:, None], bkk[:, None], bqq[:, None], mb],
                axis=1,
            )
        ).astype(np.float32)
        in_maps.append(
            {
                "qT": np.ascontiguousarray(np.asarray(query[b]).T.astype(bf16)),
                "kT": np.ascontiguousarray(np.asarray(key[b]).T.astype(bf16)),
                "vT": np.ascontiguousarray(np.asarray(value[b]).T.astype(bf16)),
                "wid": wid,
                "cst": cst,
            }
        )

    res = run_bass_kernel_spmd(
        _NC,
        in_maps,
        core_ids=list(range(8)),
        trace=bool(os.environ.get("KERNEL_TRACE")),
    )
    LAST_RESULT = res
    out = np.empty((B, S, D), dtype=np.float32)
    for b in range(B):
        arr = np.asarray(res.results[b]["o"])  # [128, 1536]
        for ci in range(NCH):
            blk = arr[(ci % 2) * D : (ci % 2) * D + D,
                      (ci // 2) * CH : (ci // 2) * CH + CH]  # O^T chunk ci
            den = arr[32 * ci, 2 * CH : 3 * CH]  # denominator row
            out[b, ci * CH : (ci + 1) * CH, :] = (blk / den[None, :]).T
    return out
